# revision 11
# baseline (speedup 1.0000x reference)
"""Trainium2 Bass kernel for nn_CapsuleLayer (grouped 5x5 capsule conv + 3-iter
dynamic routing with local softmax), data-parallel over batch N=8 across 8 cores.

Layout: spatial positions on SBUF partitions, channels on free dims.
  hw = hb*128 + p  (raster order), hb in [0,18), p in [0,128)
  uhat: [p=128, (hb=18, ci=8, do=16, co=16)] bf16.  co innermost keeps packed
  bf16 tensor_tensor ops in the DVE 2x perf mode.

Conv: host-side im2col stages tap-expanded lhsT patches in DRAM; per ci one
[128,HW] + one [72,HW] load, then per hb two matmuls (K=128, K=72) accumulate
in PSUM; evacuation is spread Act/DVE/Pool by a greedy weighted picker.

Routing restructure vs v1:
 - iteration 0 never materializes p0 = r0*S: squash stats come from S
   (nsq = r0^2 * sum_d S^2) and r0 is folded into the g2 scale, so the big
   b-update product y0 = uhat * S_b runs concurrently with the squash chain.
 - all big elementwise chains are split DVE/Pool along the innermost co dim
   (13:3 for bf16 2x ops, 11:5 for 1x ops) to balance both engines.
 - sqrt is computed as exp(+0.5*ln) so every activation (Copy/Exp/Square/Ln)
   lives in one table set -> one LoadActFuncSet total instead of nine.
 - the 5x5 spatial pools keep the PE-shift formulation (f32 shift matrices).
"""

import numpy as np
import ml_dtypes
from contextlib import ExitStack

import concourse.bass as bass
import concourse.tile as tile
from concourse import bacc, mybir
from concourse.bass_utils import run_bass_kernel_spmd

F32 = mybir.dt.float32
BF16 = mybir.dt.bfloat16
AF = mybir.ActivationFunctionType
ALU = mybir.AluOpType

CI, DI, CO, DO = 8, 8, 16, 16
H = W = 48
HW = H * W
HB = 18
ROUTING = 3

CHUNKS = [(0, 5), (5, 10), (10, 14), (14, 18)]
CODB = 13   # DVE co-share for bf16 2x add trees (Pool gets 3/16)
CODM = 12   # DVE co-share for the big products (Pool uses divide at 1.39ns/el)
CODF = 11   # DVE co-share for 1x (f32/mixed) tensor_tensor ops (Pool 5/16)
CC = 512.0  # max-pool offset: missing positions read 0 < CC-|m0|
PE_I = set()    # i-reduce chunks handled by PE identity-matmuls
PE_D = set()    # d-reduce chunks handled by PE identity-matmuls


def _emit(nc):
    p1_d = nc.dram_tensor("p1", [CI, 128, HW], BF16, kind="ExternalInput").ap()
    p2_d = nc.dram_tensor("p2", [CI, 72, HW], BF16, kind="ExternalInput").ap()
    w1_d = nc.dram_tensor("w1", [128, CI, 256], BF16, kind="ExternalInput").ap()
    w2_d = nc.dram_tensor("w2", [72, CI, 256], BF16, kind="ExternalInput").ap()
    r0_d = nc.dram_tensor("r0c", [128, HB], F32, kind="ExternalInput").ap()
    smf_d = nc.dram_tensor("smf", [128, 8, 128], F32, kind="ExternalInput").ap()
    carw_d = nc.dram_tensor("carw", [2, 2, 128], F32, kind="ExternalInput").ap()
    carf_d = nc.dram_tensor("carf", [128, 3, 128], F32, kind="ExternalInput").ap()
    chp_d = nc.dram_tensor("chp", [48, 128], F32, kind="ExternalInput").ap()
    smb_d = nc.dram_tensor("smb", [128, 4, 128], BF16, kind="ExternalInput").ap()
    cwp_d = nc.dram_tensor("cwp", [2, 3, 128], BF16, kind="ExternalInput").ap()
    carb_d = nc.dram_tensor("carb", [128, 2, 128], BF16, kind="ExternalInput").ap()
    chs_d = nc.dram_tensor("chs", [128, 2, 128], BF16, kind="ExternalInput").ap()
    idn_d = nc.dram_tensor("idn", [128, 128], BF16, kind="ExternalInput").ap()
    v_d = nc.dram_tensor("v", [128, HB, DO, CO], BF16, kind="ExternalOutput").ap()

    with tile.TileContext(nc) as tc, ExitStack() as ctx:
        const = ctx.enter_context(tc.tile_pool(name="const", bufs=1))
        patp = ctx.enter_context(tc.tile_pool(name="patp", bufs=1))
        psum = ctx.enter_context(tc.tile_pool(name="psum", bufs=8, space="PSUM"))
        big = ctx.enter_context(tc.tile_pool(name="big", bufs=1))
        scr = ctx.enter_context(tc.tile_pool(name="scr", bufs=1))
        sm = ctx.enter_context(tc.tile_pool(name="sm", bufs=1))
        poolt = ctx.enter_context(tc.tile_pool(name="poolt", bufs=1))

        # ---- persistent tiles ----
        uhat = big.tile([128, HB, CI, DO, CO], BF16, name="uhat")
        b_t = big.tile([128, HB, CI, CO], F32, name="b_t")
        p_t = big.tile([128, HB, DO, CO], BF16, name="p_t")
        v_bf = big.tile([128, HB, DO, CO], BF16, name="v_bf")
        c_t = big.tile([128, HB, CI, CO], BF16, name="c_t")
        db_t = big.tile([128, HB, CI, CO], BF16, name="db_t")
        w1_t = const.tile([128, CI, 256], BF16, name="w1_t")
        w2_t = const.tile([72, CI, 256], BF16, name="w2_t")
        r0_t = const.tile([128, HB], F32, name="r0_t")
        r0sq_t = const.tile([128, HB], F32, name="r0sq_t")
        eps_t = const.tile([128, 1], F32, name="eps_t")
        nc.sync.dma_start(w1_t[:], w1_d[:])
        nc.sync.dma_start(w2_t[:], w2_d[:])
        nc.sync.dma_start(r0_t[:], r0_d[:])
        nc.vector.memset(eps_t[:], 1e-9)
        nc.vector.tensor_tensor(r0sq_t[:], r0_t[:], r0_t[:], op=ALU.mult)

        # PE-shift pools: shift matrices (constants) + hop tiles.
        smf = const.tile([128, 8, 128], F32, name="smf")
        carw = const.tile([2, 2, 128], F32, name="carw")
        carf = const.tile([128, 3, 128], F32, name="carf")
        chp = const.tile([48, 128], F32, name="chp")
        smb = const.tile([128, 4, 128], BF16, name="smb")
        cwp = const.tile([2, 3, 128], BF16, name="cwp")
        carb = const.tile([128, 2, 128], BF16, name="carb")
        chs = const.tile([128, 2, 128], BF16, name="chs")
        idn = const.tile([128, 128], BF16, name="idn")
        nc.sync.dma_start(idn[:], idn_d[:])
        tA = poolt.tile([128, 19, CI], F32, name="tA")
        m0C_t = poolt.tile([128, 19, CI], F32, name="m0C_t")
        s_tb = poolt.tile([128, 19, CI], BF16, name="s_tb")
        nc.vector.memset(tA[:], 0.0)
        nc.vector.memset(m0C_t[:], 0.0)
        nc.vector.memset(s_tb[:], 0.0)
        # small persistent maps
        sumc_t = sm.tile([128, HB, CI], F32, name="sumc_t")
        dum = sm.tile([128, 1], F32, name="dum")
        rcp_t = sm.tile([128, HB, CI], F32, name="rcp_t")
        rcpb_t = sm.tile([128, HB, CI], BF16, name="rcpb_t")
        ysd_t = sm.tile([128, HB, CO], F32, name="ysd_t")
        cn_t = sm.tile([128, HB, CO], F32, name="cn_t")
        nsq_t = sm.tile([128, HB, CO], F32, name="nsq_t")
        rs_t = sm.tile([128, HB, CO], F32, name="rs_t")
        g2b_t = sm.tile([128, HB, CO], BF16, name="g2b_t")
        g2r0_t = sm.tile([128, HB, CO], BF16, name="g2r0_t")

        S_t = v_bf  # v_bf is free until the final iteration

        # helpers: DVE/Pool split along the hb dim (keeps co full so the
        # free dims stay <=3D after merging, an ISA requirement)
        def hb2(op, h0, h1, ph, dst_f, a_f, b_f):
            d = h1 - ph
            if d > h0:
                nc.vector.tensor_tensor(dst_f(h0, d), a_f(h0, d), b_f(h0, d),
                                        op=op)
            if ph:
                nc.gpsimd.tensor_tensor(dst_f(d, h1), a_f(d, h1), b_f(d, h1),
                                        op=op)

        # =========== Stage 1: conv -> uhat, S = sum_ci uhat ===========
        # evac engines: greedy pick by accumulated weighted load
        # DVE also carries the ~15us S accumulation during conv: bias the
        # greedy evac picker so Act takes correspondingly more copies
        ev_load = [0.0, 15000.0, 0.0]

        def evac(dst, src, sz=512.0):
            # GPSIMD cannot read PSUM on hw: rotate Act/DVE only.
            # per-elem psum-copy rates + init: Act .833/143, DVE 1.04/125
            cost = [sz * 0.833 + 143.0, sz * 1.042 + 125.0]
            e = min(range(2), key=lambda i: ev_load[i] + cost[i])
            ev_load[e] += cost[e]
            if e == 0:
                nc.scalar.copy(dst, src)
            else:
                nc.vector.tensor_copy(dst, src)

        def pe_sum(out_ps, views):
            """out_ps (PSUM f32) = sum of identically-shaped bf16 SBUF views
            via identity-matmul accumulation on the (otherwise idle) PE."""
            m = len(views) - 1
            for j, v in enumerate(views):
                nc.tensor.matmul(out_ps, idn[:], v,
                                 start=(j == 0), stop=(j == m))

        for ci in range(CI):
            pat1 = patp.tile([128, HW], BF16, name="pat1", tag="pat1", bufs=2)
            pat2 = patp.tile([72, HW], BF16, name="pat2", tag="pat2", bufs=2)
            nc.sync.dma_start(pat1[:], p1_d[ci])
            nc.sync.dma_start(pat2[:], p2_d[ci])
            for hp in range(HB // 2):
                ps = psum.tile([128, 2, 256], F32, name="ps", tag="ps", bufs=4)
                for k in range(2):
                    hb = hp * 2 + k
                    cb = hb * 128
                    lhs1 = pat1[:, cb : cb + 128]
                    lhs2 = pat2[:, cb : cb + 128]
                    nc.tensor.matmul(
                        ps[:, k], lhs1, w1_t[:, ci, :], start=True, stop=False
                    )
                    nc.tensor.matmul(
                        ps[:, k], lhs2, w2_t[:, ci, :], start=False, stop=True
                    )
                evac(uhat[:, 2 * hp : 2 * hp + 2, ci],
                     ps[:].rearrange("p k (d c) -> p k d c", d=DO))
            # running S = sum_ci uhat (hidden in conv slack)
            if ci == 0:
                nc.vector.tensor_copy(S_t[:, 0:14], uhat[:, 0:14, 0])
                nc.gpsimd.tensor_copy(S_t[:, 14:18], uhat[:, 14:18, 0])
            else:
                nc.vector.tensor_tensor(
                    S_t[:, 0:14], S_t[:, 0:14], uhat[:, 0:14, ci], op=ALU.add
                )
                nc.gpsimd.tensor_tensor(
                    S_t[:, 14:18], S_t[:, 14:18], uhat[:, 14:18, ci],
                    op=ALU.add,
                )

        # shift-matrix loads (needed first by pe_max_pools, ~80us in)
        nc.sync.dma_start(smf[:], smf_d[:])
        nc.sync.dma_start(carw[:], carw_d[:])
        nc.sync.dma_start(carf[:], carf_d[:])
        nc.sync.dma_start(chp[:], chp_d[:])
        nc.sync.dma_start(smb[:], smb_d[:])
        nc.sync.dma_start(cwp[:], cwp_d[:])
        nc.sync.dma_start(carb[:], carb_d[:])
        nc.sync.dma_start(chs[:], chs_d[:])

        # =========== PE-shift pool helpers (unchanged from v1) ===========
        def pe_max_pools():
            """m0C_t ([128,19,8] f32, m0+CC, col18=0) -> bmaxC in m0C_t."""
            cur = m0C_t
            for dst in (tA, m0C_t):  # two W hops
                psP = psum.tile([128, 19, CI], F32, name="psP", tag="pp", bufs=2)
                psM = psum.tile([128, 19, CI], F32, name="psM", tag="pp", bufs=2)
                for cls in range(3):
                    o = psP[:, cls:18:3]
                    nc.tensor.matmul(o, smf[:, cls], cur[:, cls:18:3],
                                     start=True, stop=(cls == 2))
                    if cls < 2:
                        nc.tensor.matmul(o, carw[:, cls],
                                         cur[0:2, cls + 1:19:3],
                                         start=False, stop=True)
                    o = psM[:, cls:18:3]
                    nc.tensor.matmul(o, smf[:, 3 + cls], cur[:, cls:18:3],
                                     start=True, stop=(cls == 0))
                    if cls > 0:
                        nc.tensor.matmul(o, carf[64:128, cls - 1],
                                         cur[64:128, cls - 1:18:3],
                                         start=False, stop=True)
                nc.vector.tensor_tensor(dst[:, 0:18], cur[:, 0:18],
                                        psP[:, 0:18], op=ALU.max)
                nc.vector.tensor_tensor(dst[:, 0:18], dst[:, 0:18],
                                        psM[:, 0:18], op=ALU.max)
                cur = dst
            for dst in (tA, m0C_t):  # two H hops
                psP = psum.tile([128, 19, CI], F32, name="psP", tag="pp", bufs=2)
                psM = psum.tile([128, 19, CI], F32, name="psM", tag="pp", bufs=2)
                nc.tensor.matmul(psP[:, 0:18], smf[:, 6], cur[:, 0:18],
                                 start=True, stop=False)
                nc.tensor.matmul(psP[:, 0:18], chp[:], cur[0:48, 1:19],
                                 start=False, stop=True)
                nc.tensor.matmul(psM[:, 0:18], smf[:, 7], cur[:, 0:18],
                                 start=True, stop=False)
                nc.tensor.matmul(psM[:, 1:18], carf[64:128, 2],
                                 cur[64:128, 0:17],
                                 start=False, stop=True)
                nc.vector.tensor_tensor(dst[:, 0:18], cur[:, 0:18],
                                        psP[:, 0:18], op=ALU.max)
                nc.vector.tensor_tensor(dst[:, 0:18], dst[:, 0:18],
                                        psM[:, 0:18], op=ALU.max)
                cur = dst

        def pe_sum_pools():
            """s_tb ([128,19,8] bf16, col18=0) -> sumc_t [128,18,8] f32."""
            psW = psum.tile([128, 19, CI], F32, name="psW", tag="pp", bufs=2)
            for cls in range(3):
                o = psW[:, cls:18:3]
                nc.tensor.matmul(o, smb[:, cls], s_tb[:, cls:18:3],
                                 start=True, stop=False)
                nc.tensor.matmul(o, cwp[:, cls], s_tb[0:2, cls + 1:19:3],
                                 start=False, stop=(cls == 0))
                if cls > 0:
                    nc.tensor.matmul(o, carb[64:128, cls - 1],
                                     s_tb[64:128, cls - 1:18:3],
                                     start=False, stop=True)
            nc.vector.tensor_copy(s_tb[:, 0:18], psW[:, 0:18])
            psH = psum.tile([128, 19, CI], F32, name="psH", tag="pp", bufs=2)
            nc.tensor.matmul(psH[:, 0:18], smb[:, 3], s_tb[:, 0:18],
                             start=True, stop=False)
            nc.tensor.matmul(psH[:, 0:18], chs[0:96, 0], s_tb[0:96, 1:19],
                             start=False, stop=False)
            nc.tensor.matmul(psH[:, 1:18], chs[32:64, 1], s_tb[32:64, 0:17],
                             start=False, stop=False)
            nc.tensor.matmul(psH[:, 1:18], chs[64:128, 1], s_tb[64:128, 0:17],
                             start=False, stop=True)
            nc.vector.tensor_copy(sumc_t[:], psH[:, 0:18])

        # =========== squash g-scale chain ===========
        I32 = mybir.dt.int32

        def g_chain(out_g, fold_r0, h0, h1):
            """nsq_t slice [128,h0:h1,CO] f32 -> out_g slice = squash scale
            nsq/((1+nsq)*sqrt(nsq+eps)) in bf16, times r0 if fold_r0.
            1/sqrt via the int32 exponent trick + one Newton step, all on
            DVE (keeps the single Act table pinned)."""
            nsq = nsq_t[:, h0:h1]
            rs = rs_t[:, h0:h1]
            ysd = ysd_t[:, h0:h1]
            cn = cn_t[:, h0:h1]
            nc.vector.tensor_scalar(rs, nsq, 1e-9, None, op0=ALU.add)
            yi = ysd.bitcast(I32)
            xi = rs.bitcast(I32)
            nc.vector.tensor_scalar(yi, xi, 1, None,
                                    op0=ALU.logical_shift_right)
            nc.vector.tensor_scalar(yi, yi, -1, 0x5F3759DF,
                                    op0=ALU.mult, op1=ALU.add)
            nc.vector.tensor_scalar(rs, rs, 0.5, None, op0=ALU.mult)
            nc.vector.tensor_tensor(cn, ysd, ysd, op=ALU.mult)
            nc.vector.tensor_tensor(cn, cn, rs, op=ALU.mult)
            nc.vector.tensor_scalar(cn, cn, -1.0, 1.5,
                                    op0=ALU.mult, op1=ALU.add)
            nc.vector.tensor_tensor(ysd, ysd, cn, op=ALU.mult)
            # g = nsq * rsqrt(nsq+eps) / (1+nsq)
            nc.vector.tensor_scalar(rs, nsq, 1.0, None, op0=ALU.add)
            nc.vector.reciprocal(rs, rs)
            nc.vector.tensor_tensor(ysd, ysd, nsq, op=ALU.mult)
            if fold_r0:
                nc.vector.tensor_tensor(ysd, ysd, rs, op=ALU.mult)
                r0_b = r0_t[:, h0:h1].unsqueeze(2).broadcast_to(
                    [128, h1 - h0, CO])
                nc.vector.tensor_tensor(out_g[:, h0:h1], ysd, r0_b,
                                        op=ALU.mult)
            else:
                nc.vector.tensor_tensor(out_g[:, h0:h1], ysd, rs,
                                        op=ALU.mult)

        # squash stat tree, split in two parts: sq on Act (emitted with its
        # chunk), the small DVE n-levels emitted later so they never stall
        # the big-product pipeline
        def nsq_sq(src_t, h0, h1):
            n = h1 - h0
            sq = scr.tile([128, n, DO, CO], BF16, name="sq", tag="SQ", bufs=3)
            nc.scalar.activation(sq[:], src_t[:, h0:h1], AF.Square)
            return sq

        def nsq_levels(sq, h0, h1):
            n = h1 - h0
            n1 = scr.tile([128, n, 8, CO], BF16, name="n1", tag="N1", bufs=1)
            nc.vector.tensor_tensor(n1[:], sq[:, :, 0:8], sq[:, :, 8:16],
                                    op=ALU.add)
            n2 = scr.tile([128, n, 4, CO], BF16, name="n2", tag="N2", bufs=1)
            nc.vector.tensor_tensor(n2[:], n1[:, :, 0:4], n1[:, :, 4:8],
                                    op=ALU.add)
            n3 = scr.tile([128, n, 2, CO], BF16, name="n3", tag="N3", bufs=1)
            nc.vector.tensor_tensor(n3[:], n2[:, :, 0:2], n2[:, :, 2:4],
                                    op=ALU.add)
            nc.vector.tensor_tensor(nsq_t[:, h0:h1], n3[:, :, 0], n3[:, :, 1],
                                    op=ALU.add)

        # big product + d-tree: db_t[:, h0:h1] = sum_d uhat * m_b  (m bf16
        # [128,HB,DO,CO] broadcast over ci)
        def d_contract(m_t, rcm, h0, h1):
            n = h1 - h0
            m_b = m_t[:].unsqueeze(2).broadcast_to([128, HB, CI, DO, CO])
            y = scr.tile([128, n, CI, DO, CO], BF16, name="y", tag="X")
            hb2(ALU.mult, h0, h1, 1,
                lambda a, b: y[:, a - h0 : b - h0],
                lambda a, b: uhat[:, a:b],
                lambda a, b: m_b[:, a:b])
            if h0 in PE_D:
                for g0 in range(0, n, 2):
                    g1 = min(g0 + 2, n)
                    dps = psum.tile([128, 2, CI, CO], F32, name="dps",
                                    tag="pr", bufs=2)
                    o = dps[:, 0 : g1 - g0]
                    pe_sum(o, [y[:, g0:g1, :, d] for d in range(DO)])
                    evac(db_t[:, h0 + g0 : h0 + g1], o,
                         sz=(g1 - g0) * 128.0)
                return
            e1 = scr.tile([128, n, CI, 8, CO], BF16, name="e1", tag="T1")
            hb2(ALU.add, h0, h1, 1,
                lambda a, b: e1[:, a - h0 : b - h0],
                lambda a, b: y[:, a - h0 : b - h0, :, 0:8],
                lambda a, b: y[:, a - h0 : b - h0, :, 8:16])
            e2 = scr.tile([128, n, CI, 4, CO], BF16, name="e2", tag="T2")
            hb2(ALU.add, h0, h1, 1,
                lambda a, b: e2[:, a - h0 : b - h0],
                lambda a, b: e1[:, a - h0 : b - h0, :, 0:4],
                lambda a, b: e1[:, a - h0 : b - h0, :, 4:8])
            e3 = scr.tile([128, n, CI, 2, CO], BF16, name="e3", tag="T3")
            hb2(ALU.add, h0, h1, 1,
                lambda a, b: e3[:, a - h0 : b - h0],
                lambda a, b: e2[:, a - h0 : b - h0, :, 0:2],
                lambda a, b: e2[:, a - h0 : b - h0, :, 2:4])
            hb2(ALU.add, h0, h1, 1,
                lambda a, b: db_t[:, a:b],
                lambda a, b: e3[:, a - h0 : b - h0, :, 0],
                lambda a, b: e3[:, a - h0 : b - h0, :, 1])

        # i-contract: p_t[:, h0:h1] = sum_i c_t * uhat  (c_t bf16 weights)
        def i_contract(h0, h1):
            n = h1 - h0
            r_b = c_t[:].unsqueeze(3).broadcast_to([128, HB, CI, DO, CO])
            x = scr.tile([128, n, CI, DO, CO], BF16, name="x", tag="X")
            hb2(ALU.mult, h0, h1, 1,
                lambda a, b: x[:, a - h0 : b - h0],
                lambda a, b: uhat[:, a:b],
                lambda a, b: r_b[:, a:b])
            if h0 in PE_I:
                for g0 in range(0, n, 2):
                    g1 = min(g0 + 2, n)
                    pps = psum.tile([128, 2, DO, CO], F32, name="pps",
                                    tag="pr", bufs=2)
                    o = pps[:, 0 : g1 - g0]
                    pe_sum(o, [x[:, g0:g1, i] for i in range(CI)])
                    evac(p_t[:, h0 + g0 : h0 + g1], o,
                         sz=(g1 - g0) * 256.0)
                return
            t1 = scr.tile([128, n, 4, DO, CO], BF16, name="t1", tag="T1")
            hb2(ALU.add, h0, h1, 1,
                lambda a, b: t1[:, a - h0 : b - h0],
                lambda a, b: x[:, a - h0 : b - h0, 0:4],
                lambda a, b: x[:, a - h0 : b - h0, 4:8])
            t2 = scr.tile([128, n, 2, DO, CO], BF16, name="t2", tag="T2")
            hb2(ALU.add, h0, h1, 1,
                lambda a, b: t2[:, a - h0 : b - h0],
                lambda a, b: t1[:, a - h0 : b - h0, 0:2],
                lambda a, b: t1[:, a - h0 : b - h0, 2:4])
            hb2(ALU.add, h0, h1, 1,
                lambda a, b: p_t[:, a:b],
                lambda a, b: t2[:, a - h0 : b - h0, 0],
                lambda a, b: t2[:, a - h0 : b - h0, 1])

        # b max-tree (over co) -> m0C chunk (+CC)
        def u_tree(h0, h1):
            n = h1 - h0
            u1 = scr.tile([128, n, CI, 8], F32, name="u1", tag="U1", bufs=1)
            nc.vector.tensor_tensor(
                u1[:], b_t[:, h0:h1, :, 0:8],
                b_t[:, h0:h1, :, 8:16], op=ALU.max)
            u2 = scr.tile([128, n, CI, 4], F32, name="u2", tag="U2", bufs=1)
            nc.vector.tensor_tensor(u2[:], u1[:, :, :, 0:4], u1[:, :, :, 4:8],
                                    op=ALU.max)
            u3 = scr.tile([128, n, CI, 2], F32, name="u3", tag="U3", bufs=1)
            nc.vector.tensor_tensor(u3[:], u2[:, :, :, 0:2], u2[:, :, :, 2:4],
                                    op=ALU.max)
            nc.vector.tensor_tensor(m0C_t[:, h0:h1], u3[:, :, :, 0],
                                    u3[:, :, :, 1], op=ALU.max)
            nc.vector.tensor_scalar(m0C_t[:, h0:h1], m0C_t[:, h0:h1], CC,
                                    None, op0=ALU.add)

        # =========== Stage 2 it0: b1 = (g0*r0) .* (sum_d uhat*S) ===========
        # big products first (only need S), squash stats run concurrently
        sqs = {}
        for (h0, h1) in CHUNKS:
            d_contract(S_t, None, h0, h1)   # y0 = uhat*S_b, tree -> db_t
            sqs[h0] = nsq_sq(S_t, h0, h1)   # Act squares, no DVE stall
        for (h0, h1) in CHUNKS:
            nsq_levels(sqs[h0], h0, h1)
        r0sq_b = r0sq_t[:].unsqueeze(2).broadcast_to([128, HB, CO])
        nc.vector.tensor_tensor(nsq_t[:], nsq_t[:], r0sq_b, op=ALU.mult)
        g_chain(g2r0_t, True, 0, HB)
        g_b0 = g2r0_t[:].unsqueeze(2).broadcast_to([128, HB, CI, CO])
        for (h0, h1) in CHUNKS:
            hb2(ALU.mult, h0, h1, 2,
                lambda a, b: b_t[:, a:b],
                lambda a, b: db_t[:, a:b],
                lambda a, b: g_b0[:, a:b])
            u_tree(h0, h1)

        # =========== Stage 2 it1/it2 ===========
        for it in (1, 2):
            last = it == ROUTING - 1
            pe_max_pools()
            # cs = (bmaxC - CC) - b = -(b - bmax); exp applies scale=-1
            nc.vector.tensor_scalar(
                tA[:, 0:18], m0C_t[:, 0:18], -CC, None, op0=ALU.add)
            cs = scr.tile([128, HB, CI, CO], F32, name="cs", tag="X")
            bm_b = tA[:, 0:18].unsqueeze(3).broadcast_to([128, HB, CI, CO])
            for (e0, e1_) in ((0, 6), (6, 12), (12, 18)):
                hb2(ALU.subtract, e0, e1_, 2,
                    lambda a, b: cs[:, a:b],
                    lambda a, b: bm_b[:, a:b],
                    lambda a, b: b_t[:, a:b])
                nc.scalar.activation(c_t[:, e0:e1_], cs[:, e0:e1_], AF.Exp,
                                     scale=-1.0)
            s1 = scr.tile([128, HB, CI, 8], BF16, name="s1", tag="S1")
            nc.vector.tensor_tensor(
                s1[:], c_t[:, :, :, 0:8], c_t[:, :, :, 8:16], op=ALU.add)
            s2 = scr.tile([128, HB, CI, 4], BF16, name="s2", tag="S2")
            nc.vector.tensor_tensor(
                s2[:], s1[:, :, :, 0:4], s1[:, :, :, 4:8], op=ALU.add)
            s3 = scr.tile([128, HB, CI, 2], BF16, name="s3", tag="S3")
            nc.vector.tensor_tensor(
                s3[:], s2[:, :, :, 0:2], s2[:, :, :, 2:4], op=ALU.add)
            nc.vector.tensor_tensor(
                s_tb[:, 0:18], s3[:, :, :, 0], s3[:, :, :, 1], op=ALU.add)
            pe_sum_pools()
            nc.vector.reciprocal(rcp_t[:], sumc_t[:])
            nc.vector.tensor_copy(rcpb_t[:], rcp_t[:])
            rb_ = rcpb_t[:].unsqueeze(3).broadcast_to([128, HB, CI, CO])
            hb2(ALU.mult, 0, HB, 4,
                lambda a, b: c_t[:, a:b],
                lambda a, b: c_t[:, a:b],
                lambda a, b: rb_[:, a:b])
            # p = sum_i r*uhat ; squash stats per chunk on Act
            sqs = {}
            for (h0, h1) in CHUNKS:
                i_contract(h0, h1)
                sqs[h0] = nsq_sq(p_t, h0, h1)
            if not last:
                for (h0, h1) in CHUNKS:
                    d_contract(p_t, None, h0, h1)
                for (h0, h1) in CHUNKS:
                    nsq_levels(sqs[h0], h0, h1)
                g_chain(g2b_t, False, 0, HB)
                g_b1 = g2b_t[:].unsqueeze(2).broadcast_to(
                    [128, HB, CI, CO])
                for (h0, h1) in CHUNKS:
                    hb2(ALU.mult, h0, h1, 1,
                        lambda a, b: db_t[:, a:b],
                        lambda a, b: db_t[:, a:b],
                        lambda a, b: g_b1[:, a:b])
                    hb2(ALU.add, h0, h1, 2,
                        lambda a, b: b_t[:, a:b],
                        lambda a, b: b_t[:, a:b],
                        lambda a, b: db_t[:, a:b])
                    u_tree(h0, h1)
            else:
                for (hh0, hh1) in ((0, 10), (10, 18)):
                    for (h0, h1) in CHUNKS:
                        if h0 < hh0 or h1 > hh1:
                            continue
                        nsq_levels(sqs[h0], h0, h1)
                    g_chain(g2b_t, False, hh0, hh1)
                    for (h0, h1) in CHUNKS:
                        if h0 < hh0 or h1 > hh1:
                            continue
                        g_b2 = g2b_t[:].unsqueeze(2).broadcast_to(
                            [128, HB, DO, CO])
                        hb2(ALU.mult, h0, h1, 1,
                            lambda a, b: v_bf[:, a:b],
                            lambda a, b: p_t[:, a:b],
                            lambda a, b: g_b2[:, a:b])
                        nc.sync.dma_start(v_d[:, h0:h1], v_bf[:, h0:h1])
    return nc


# ============================ host side ============================

_CACHE = {}


def _host_consts(w):
    # w: [Ci, Co*Do, Di, 5, 5] f32, channel index = co*16+do.
    # Conv lhsT rows: pat1 row = di*16 + kh*4 + kw (kh,kw in 0..4);
    # pat2 row = di*5 + kw for (kh=4, kw 0..5), then 40 + di*4 + kh for
    # (kh 0..4, kw=4).  Columns m = do*16 + co.
    w4 = w.reshape(CI, CO, DO, DI, 5, 5).transpose(3, 4, 5, 0, 2, 1)
    # w4: [di, kh, kw, ci, do, co]
    w4 = np.ascontiguousarray(w4).reshape(DI, 5, 5, CI, 256)
    w1 = np.ascontiguousarray(
        w4[:, 0:4, 0:4].reshape(128, CI, 256)
    ).astype(ml_dtypes.bfloat16)
    w2a = w4[:, 4, 0:5].reshape(40, CI, 256)
    w2b = w4[:, 0:4, 4].reshape(32, CI, 256)
    w2 = np.ascontiguousarray(np.concatenate([w2a, w2b], 0)).astype(
        ml_dtypes.bfloat16
    )

    hw_cnt = np.zeros((H, W), np.float32)
    for h in range(H):
        for wv in range(W):
            ch = min(h + 2, H - 1) - max(h - 2, 0) + 1
            cw = min(wv + 2, W - 1) - max(wv - 2, 0) + 1
            hw_cnt[h, wv] = ch * cw
    r0 = 1.0 / (CO * hw_cnt)
    r0c = np.ascontiguousarray(r0.reshape(HB, 128).T)
    return w1, w2, r0c


def _shift_mats():
    """Constant PE shift matrices for the 5x5 window pools (see _emit)."""
    def wof(cls, p):
        return (32 * cls + p) % 48

    smf = np.zeros((128, 8, 128), np.float32)
    for cls in range(3):
        for m in range(128):
            w = wof(cls, m)
            if m + 1 < 128 and w < 47:
                smf[m + 1, cls, m] = 1.0
            if m - 1 >= 0 and w >= 1:
                smf[m - 1, 3 + cls, m] = 1.0
    for m in range(80):
        smf[m + 48, 6, m] = 1.0
    for m in range(48, 128):
        smf[m - 48, 7, m] = 1.0

    carw = np.zeros((2, 2, 128), np.float32)
    carw[0, 0, 127] = 1.0  # c1p cls0 (w(127)=31 valid)
    carw[0, 1, 127] = 1.0  # c1p cls1 (w(127)=15 valid)

    carf = np.zeros((128, 3, 128), np.float32)
    carf[127, 0, 0] = 1.0  # c1m cls1 (w(0)=32 valid)
    carf[127, 1, 0] = 1.0  # c1m cls2 (w(0)=16 valid)
    for m in range(48):    # chm: out[m] = in_prev[m+80]
        carf[m + 80, 2, m] = 1.0

    chp = np.zeros((48, 128), np.float32)
    for m in range(80, 128):
        chp[m - 80, m] = 1.0

    smb = np.zeros((128, 4, 128), np.float32)
    for cls in range(3):
        for m in range(128):
            for dw in range(-2, 3):
                k = m + dw
                if 0 <= k < 128 and 0 <= wof(cls, m) + dw < 48:
                    smb[k, cls, m] = 1.0
    for m in range(128):
        for dh in range(-2, 3):
            k = m + 48 * dh
            if 0 <= k < 128:
                smb[k, 3, m] = 1.0

    cwp = np.zeros((2, 3, 128), np.float32)
    carb = np.zeros((128, 2, 128), np.float32)
    for cls in range(3):
        for m in range(126, 128):
            for dw in (1, 2):
                k = m + dw - 128
                if 0 <= k < 2 and wof(cls, m) + dw < 48:
                    cwp[k, cls, m] = 1.0
        if cls > 0:
            for m in range(0, 2):
                for dw in (-2, -1):
                    k = m + dw + 128
                    if 126 <= k < 128 and wof(cls, m) + dw >= 0:
                        carb[k, cls - 1, m] = 1.0

    chs = np.zeros((128, 2, 128), np.float32)
    for m in range(80, 128):
        chs[m - 80, 0, m] += 1.0
    for m in range(32, 128):
        chs[m - 32, 0, m] += 1.0
    for m in range(48):
        chs[m + 80, 1, m] += 1.0
    for m in range(96):
        chs[m + 32, 1, m] += 1.0

    bf = ml_dtypes.bfloat16
    return (smf, carw, carf, chp, smb.astype(bf), cwp.astype(bf),
            carb.astype(bf), chs.astype(bf))


def _im2col(un):
    """un: [Ci, Di, H, W] bf16 -> pat1 [Ci, 128, HW], pat2 [Ci, 72, HW] bf16.
    Row layouts match _host_consts."""
    up = np.zeros((CI, DI, H + 4, W + 4), ml_dtypes.bfloat16)
    up[:, :, 2 : 2 + H, 2 : 2 + W] = un
    sw = np.lib.stride_tricks.sliding_window_view(up, (H, W), axis=(2, 3))
    # sw: [Ci, Di, 5, 5, H, W]
    p1 = sw[:, :, 0:4, 0:4].reshape(CI, DI * 16, HW)
    p2a = sw[:, :, 4, 0:5].reshape(CI, DI * 5, HW)
    p2b = sw[:, :, 0:4, 4].reshape(CI, DI * 4, HW)
    p2 = np.concatenate([p2a, p2b], 1)
    return np.ascontiguousarray(p1), np.ascontiguousarray(p2)


def _get_nc():
    if "nc" not in _CACHE:
        nc = bacc.Bacc("TRN2", target_bir_lowering=False, debug=False, num_devices=8)
        _emit(nc)
        nc.compile()
        _CACHE["nc"] = nc
    return _CACHE["nc"]


def kernel(u, w):
    u = np.asarray(u, np.float32)
    N = u.shape[0]
    assert N == 8
    nc = _get_nc()
    w1, w2, r0c = _host_consts(np.asarray(w, np.float32))
    smf, carw, carf, chp, smb, cwp, carb, chs = _shift_mats()
    idn = np.eye(128, dtype=ml_dtypes.bfloat16)
    ub = u.astype(ml_dtypes.bfloat16)
    in_maps = []
    for n in range(N):
        p1, p2 = _im2col(ub[n])
        in_maps.append({"p1": p1, "p2": p2, "w1": w1, "w2": w2, "r0c": r0c,
                        "smf": smf, "carw": carw, "carf": carf, "chp": chp,
                        "smb": smb, "cwp": cwp, "carb": carb, "chs": chs,
                        "idn": idn})
    res = run_bass_kernel_spmd(nc, in_maps, core_ids=list(range(N)))
    out = np.stack(
        [res.results[n]["v"].astype(np.float32) for n in range(N)]
    )  # [8, 128, HB, DO, CO]
    # hw = hb*128 + p ; out[n, co, do, h, w]
    out = out.transpose(0, 2, 1, 3, 4).reshape(N, HW, DO, CO)
    out = out.reshape(N, H, W, DO, CO).transpose(0, 4, 3, 1, 2)
    return np.ascontiguousarray(out, dtype=np.float32)


# revision 12
# speedup vs baseline: 1.0044x; 1.0044x over previous
"""Trainium2 Bass kernel for nn_CapsuleLayer (grouped 5x5 capsule conv + 3-iter
dynamic routing with local softmax), data-parallel over batch N=8 across 8 cores.

Layout: spatial positions on SBUF partitions, channels on free dims.
  hw = hb*128 + p  (raster order), hb in [0,18), p in [0,128)
  uhat: [p=128, (hb=18, ci=8, do=16, co=16)] bf16.  co innermost keeps packed
  bf16 tensor_tensor ops in the DVE 2x perf mode.

Conv: host-side im2col stages tap-expanded lhsT patches in DRAM; per ci one
[128,HW] + one [72,HW] load, then per hb two matmuls (K=128, K=72) accumulate
in PSUM; evacuation is spread Act/DVE/Pool by a greedy weighted picker.

Routing restructure vs v1:
 - iteration 0 never materializes p0 = r0*S: squash stats come from S
   (nsq = r0^2 * sum_d S^2) and r0 is folded into the g2 scale, so the big
   b-update product y0 = uhat * S_b runs concurrently with the squash chain.
 - all big elementwise chains are split DVE/Pool along the innermost co dim
   (13:3 for bf16 2x ops, 11:5 for 1x ops) to balance both engines.
 - sqrt is computed as exp(+0.5*ln) so every activation (Copy/Exp/Square/Ln)
   lives in one table set -> one LoadActFuncSet total instead of nine.
 - the 5x5 spatial pools keep the PE-shift formulation (f32 shift matrices).
"""

import numpy as np
import ml_dtypes
from contextlib import ExitStack

import concourse.bass as bass
import concourse.tile as tile
from concourse import bacc, mybir
from concourse.bass_utils import run_bass_kernel_spmd

F32 = mybir.dt.float32
BF16 = mybir.dt.bfloat16
AF = mybir.ActivationFunctionType
ALU = mybir.AluOpType

CI, DI, CO, DO = 8, 8, 16, 16
H = W = 48
HW = H * W
HB = 18
ROUTING = 3

CHUNKS = [(0, 5), (5, 10), (10, 14), (14, 18)]
CODB = 13   # DVE co-share for bf16 2x add trees (Pool gets 3/16)
CODM = 12   # DVE co-share for the big products (Pool uses divide at 1.39ns/el)
CODF = 11   # DVE co-share for 1x (f32/mixed) tensor_tensor ops (Pool 5/16)
CC = 512.0  # max-pool offset: missing positions read 0 < CC-|m0|
PE_I = set()    # i-reduce chunks handled by PE identity-matmuls
PE_D = set()    # d-reduce chunks handled by PE identity-matmuls


def _emit(nc):
    p1_d = nc.dram_tensor("p1", [CI, 128, HW], BF16, kind="ExternalInput").ap()
    p2_d = nc.dram_tensor("p2", [CI, 72, HW], BF16, kind="ExternalInput").ap()
    w1_d = nc.dram_tensor("w1", [128, CI, 256], BF16, kind="ExternalInput").ap()
    w2_d = nc.dram_tensor("w2", [72, CI, 256], BF16, kind="ExternalInput").ap()
    r0_d = nc.dram_tensor("r0c", [128, HB], F32, kind="ExternalInput").ap()
    smf_d = nc.dram_tensor("smf", [128, 8, 128], F32, kind="ExternalInput").ap()
    carw_d = nc.dram_tensor("carw", [2, 2, 128], F32, kind="ExternalInput").ap()
    carf_d = nc.dram_tensor("carf", [128, 3, 128], F32, kind="ExternalInput").ap()
    chp_d = nc.dram_tensor("chp", [48, 128], F32, kind="ExternalInput").ap()
    smb_d = nc.dram_tensor("smb", [128, 4, 128], BF16, kind="ExternalInput").ap()
    cwp_d = nc.dram_tensor("cwp", [2, 3, 128], BF16, kind="ExternalInput").ap()
    carb_d = nc.dram_tensor("carb", [128, 2, 128], BF16, kind="ExternalInput").ap()
    chs_d = nc.dram_tensor("chs", [128, 2, 128], BF16, kind="ExternalInput").ap()
    idn_d = nc.dram_tensor("idn", [128, 128], BF16, kind="ExternalInput").ap()
    v_d = nc.dram_tensor("v", [128, HB, DO, CO], BF16, kind="ExternalOutput").ap()

    with tile.TileContext(nc) as tc, ExitStack() as ctx:
        const = ctx.enter_context(tc.tile_pool(name="const", bufs=1))
        patp = ctx.enter_context(tc.tile_pool(name="patp", bufs=1))
        psum = ctx.enter_context(tc.tile_pool(name="psum", bufs=8, space="PSUM"))
        big = ctx.enter_context(tc.tile_pool(name="big", bufs=1))
        scr = ctx.enter_context(tc.tile_pool(name="scr", bufs=1))
        sm = ctx.enter_context(tc.tile_pool(name="sm", bufs=1))
        poolt = ctx.enter_context(tc.tile_pool(name="poolt", bufs=1))

        # ---- persistent tiles ----
        uhat = big.tile([128, HB, CI, DO, CO], BF16, name="uhat")
        b_t = big.tile([128, HB, CI, CO], F32, name="b_t")
        p_t = big.tile([128, HB, DO, CO], BF16, name="p_t")
        v_bf = big.tile([128, HB, DO, CO], BF16, name="v_bf")
        c_t = big.tile([128, HB, CI, CO], BF16, name="c_t")
        db_t = big.tile([128, HB, CI, CO], BF16, name="db_t")
        w1_t = const.tile([128, CI, 256], BF16, name="w1_t")
        w2_t = const.tile([72, CI, 256], BF16, name="w2_t")
        r0_t = const.tile([128, HB], F32, name="r0_t")
        r0sq_t = const.tile([128, HB], F32, name="r0sq_t")
        eps_t = const.tile([128, 1], F32, name="eps_t")
        nc.sync.dma_start(w1_t[:], w1_d[:])
        nc.sync.dma_start(w2_t[:], w2_d[:])
        nc.sync.dma_start(r0_t[:], r0_d[:])
        nc.vector.memset(eps_t[:], 1e-9)
        nc.vector.tensor_tensor(r0sq_t[:], r0_t[:], r0_t[:], op=ALU.mult)

        # PE-shift pools: shift matrices (constants) + hop tiles.
        smf = const.tile([128, 8, 128], F32, name="smf")
        carw = const.tile([2, 2, 128], F32, name="carw")
        carf = const.tile([128, 3, 128], F32, name="carf")
        chp = const.tile([48, 128], F32, name="chp")
        smb = const.tile([128, 4, 128], BF16, name="smb")
        cwp = const.tile([2, 3, 128], BF16, name="cwp")
        carb = const.tile([128, 2, 128], BF16, name="carb")
        chs = const.tile([128, 2, 128], BF16, name="chs")
        idn = const.tile([128, 128], BF16, name="idn")
        nc.sync.dma_start(idn[:], idn_d[:])
        tA = poolt.tile([128, 19, CI], F32, name="tA")
        m0C_t = poolt.tile([128, 19, CI], F32, name="m0C_t")
        s_tb = poolt.tile([128, 19, CI], BF16, name="s_tb")
        nc.vector.memset(tA[:], 0.0)
        nc.vector.memset(m0C_t[:], 0.0)
        nc.vector.memset(s_tb[:], 0.0)
        # small persistent maps
        sumc_t = sm.tile([128, HB, CI], F32, name="sumc_t")
        dum = sm.tile([128, 1], F32, name="dum")
        rcp_t = sm.tile([128, HB, CI], F32, name="rcp_t")
        rcpb_t = sm.tile([128, HB, CI], BF16, name="rcpb_t")
        ysd_t = sm.tile([128, HB, CO], F32, name="ysd_t")
        cn_t = sm.tile([128, HB, CO], F32, name="cn_t")
        nsq_t = sm.tile([128, HB, CO], F32, name="nsq_t")
        rs_t = sm.tile([128, HB, CO], F32, name="rs_t")
        g2b_t = sm.tile([128, HB, CO], BF16, name="g2b_t")
        g2r0_t = sm.tile([128, HB, CO], BF16, name="g2r0_t")

        S_t = v_bf  # v_bf is free until the final iteration

        # helpers: DVE/Pool split along the hb dim (keeps co full so the
        # free dims stay <=3D after merging, an ISA requirement)
        def hb2(op, h0, h1, ph, dst_f, a_f, b_f):
            d = h1 - ph
            if d > h0:
                nc.vector.tensor_tensor(dst_f(h0, d), a_f(h0, d), b_f(h0, d),
                                        op=op)
            if ph:
                nc.gpsimd.tensor_tensor(dst_f(d, h1), a_f(d, h1), b_f(d, h1),
                                        op=op)

        # =========== Stage 1: conv -> uhat, S = sum_ci uhat ===========
        # evac engines: greedy pick by accumulated weighted load
        # DVE also carries the ~15us S accumulation during conv: bias the
        # greedy evac picker so Act takes correspondingly more copies
        ev_load = [0.0, 8000.0, 0.0]

        def evac(dst, src, sz=512.0):
            # GPSIMD cannot read PSUM on hw: rotate Act/DVE only.
            # per-elem psum-copy rates + init: Act .833/143, DVE 1.04/125
            cost = [sz * 0.833 + 143.0, sz * 1.042 + 125.0]
            e = min(range(2), key=lambda i: ev_load[i] + cost[i])
            ev_load[e] += cost[e]
            if e == 0:
                nc.scalar.copy(dst, src)
            else:
                nc.vector.tensor_copy(dst, src)

        def pe_sum(out_ps, views):
            """out_ps (PSUM f32) = sum of identically-shaped bf16 SBUF views
            via identity-matmul accumulation on the (otherwise idle) PE."""
            m = len(views) - 1
            for j, v in enumerate(views):
                nc.tensor.matmul(out_ps, idn[:], v,
                                 start=(j == 0), stop=(j == m))

        for ci in range(CI):
            pat1 = patp.tile([128, HW], BF16, name="pat1", tag="pat1", bufs=2)
            pat2 = patp.tile([72, HW], BF16, name="pat2", tag="pat2", bufs=2)
            nc.sync.dma_start(pat1[:], p1_d[ci])
            nc.sync.dma_start(pat2[:], p2_d[ci])
            for hp in range(HB // 2):
                ps = psum.tile([128, 2, 256], F32, name="ps", tag="ps", bufs=4)
                for k in range(2):
                    hb = hp * 2 + k
                    cb = hb * 128
                    lhs1 = pat1[:, cb : cb + 128]
                    lhs2 = pat2[:, cb : cb + 128]
                    nc.tensor.matmul(
                        ps[:, k], lhs1, w1_t[:, ci, :], start=True, stop=False
                    )
                    nc.tensor.matmul(
                        ps[:, k], lhs2, w2_t[:, ci, :], start=False, stop=True
                    )
                evac(uhat[:, 2 * hp : 2 * hp + 2, ci],
                     ps[:].rearrange("p k (d c) -> p k d c", d=DO))
            # running S = sum_ci uhat (hidden in conv slack)
            if ci == 0:
                nc.vector.tensor_copy(S_t[:, 0:14], uhat[:, 0:14, 0])
                nc.gpsimd.tensor_copy(S_t[:, 14:18], uhat[:, 14:18, 0])
            else:
                nc.vector.tensor_tensor(
                    S_t[:, 0:14], S_t[:, 0:14], uhat[:, 0:14, ci], op=ALU.add
                )
                nc.gpsimd.tensor_tensor(
                    S_t[:, 14:18], S_t[:, 14:18], uhat[:, 14:18, ci],
                    op=ALU.add,
                )

        # shift-matrix loads (needed first by pe_max_pools, ~80us in)
        nc.sync.dma_start(smf[:], smf_d[:])
        nc.sync.dma_start(carw[:], carw_d[:])
        nc.sync.dma_start(carf[:], carf_d[:])
        nc.sync.dma_start(chp[:], chp_d[:])
        nc.sync.dma_start(smb[:], smb_d[:])
        nc.sync.dma_start(cwp[:], cwp_d[:])
        nc.sync.dma_start(carb[:], carb_d[:])
        nc.sync.dma_start(chs[:], chs_d[:])

        # =========== PE-shift pool helpers (unchanged from v1) ===========
        def pe_max_pools():
            """m0C_t ([128,19,8] f32, m0+CC, col18=0) -> bmaxC in m0C_t."""
            cur = m0C_t
            for dst in (tA, m0C_t):  # two W hops
                psP = psum.tile([128, 19, CI], F32, name="psP", tag="pp", bufs=2)
                psM = psum.tile([128, 19, CI], F32, name="psM", tag="pp", bufs=2)
                for cls in range(3):
                    o = psP[:, cls:18:3]
                    nc.tensor.matmul(o, smf[:, cls], cur[:, cls:18:3],
                                     start=True, stop=(cls == 2))
                    if cls < 2:
                        nc.tensor.matmul(o, carw[:, cls],
                                         cur[0:2, cls + 1:19:3],
                                         start=False, stop=True)
                    o = psM[:, cls:18:3]
                    nc.tensor.matmul(o, smf[:, 3 + cls], cur[:, cls:18:3],
                                     start=True, stop=(cls == 0))
                    if cls > 0:
                        nc.tensor.matmul(o, carf[64:128, cls - 1],
                                         cur[64:128, cls - 1:18:3],
                                         start=False, stop=True)
                nc.vector.tensor_tensor(dst[:, 0:18], cur[:, 0:18],
                                        psP[:, 0:18], op=ALU.max)
                nc.vector.tensor_tensor(dst[:, 0:18], dst[:, 0:18],
                                        psM[:, 0:18], op=ALU.max)
                cur = dst
            for dst in (tA, m0C_t):  # two H hops
                psP = psum.tile([128, 19, CI], F32, name="psP", tag="pp", bufs=2)
                psM = psum.tile([128, 19, CI], F32, name="psM", tag="pp", bufs=2)
                nc.tensor.matmul(psP[:, 0:18], smf[:, 6], cur[:, 0:18],
                                 start=True, stop=False)
                nc.tensor.matmul(psP[:, 0:18], chp[:], cur[0:48, 1:19],
                                 start=False, stop=True)
                nc.tensor.matmul(psM[:, 0:18], smf[:, 7], cur[:, 0:18],
                                 start=True, stop=False)
                nc.tensor.matmul(psM[:, 1:18], carf[64:128, 2],
                                 cur[64:128, 0:17],
                                 start=False, stop=True)
                nc.vector.tensor_tensor(dst[:, 0:18], cur[:, 0:18],
                                        psP[:, 0:18], op=ALU.max)
                nc.vector.tensor_tensor(dst[:, 0:18], dst[:, 0:18],
                                        psM[:, 0:18], op=ALU.max)
                cur = dst

        def pe_sum_pools():
            """s_tb ([128,19,8] bf16, col18=0) -> sumc_t [128,18,8] f32."""
            psW = psum.tile([128, 19, CI], F32, name="psW", tag="pp", bufs=2)
            for cls in range(3):
                o = psW[:, cls:18:3]
                nc.tensor.matmul(o, smb[:, cls], s_tb[:, cls:18:3],
                                 start=True, stop=False)
                nc.tensor.matmul(o, cwp[:, cls], s_tb[0:2, cls + 1:19:3],
                                 start=False, stop=(cls == 0))
                if cls > 0:
                    nc.tensor.matmul(o, carb[64:128, cls - 1],
                                     s_tb[64:128, cls - 1:18:3],
                                     start=False, stop=True)
            nc.vector.tensor_copy(s_tb[:, 0:18], psW[:, 0:18])
            psH = psum.tile([128, 19, CI], F32, name="psH", tag="pp", bufs=2)
            nc.tensor.matmul(psH[:, 0:18], smb[:, 3], s_tb[:, 0:18],
                             start=True, stop=False)
            nc.tensor.matmul(psH[:, 0:18], chs[0:96, 0], s_tb[0:96, 1:19],
                             start=False, stop=False)
            nc.tensor.matmul(psH[:, 1:18], chs[32:64, 1], s_tb[32:64, 0:17],
                             start=False, stop=False)
            nc.tensor.matmul(psH[:, 1:18], chs[64:128, 1], s_tb[64:128, 0:17],
                             start=False, stop=True)
            nc.vector.tensor_copy(sumc_t[:], psH[:, 0:18])

        # =========== squash g-scale chain ===========
        I32 = mybir.dt.int32

        def g_chain(out_g, fold_r0, h0, h1):
            """nsq_t slice [128,h0:h1,CO] f32 -> out_g slice = squash scale
            nsq/((1+nsq)*sqrt(nsq+eps)) in bf16, times r0 if fold_r0.
            1/sqrt via the int32 exponent trick + one Newton step, all on
            DVE (keeps the single Act table pinned)."""
            nsq = nsq_t[:, h0:h1]
            rs = rs_t[:, h0:h1]
            ysd = ysd_t[:, h0:h1]
            cn = cn_t[:, h0:h1]
            nc.vector.tensor_scalar(rs, nsq, 1e-9, None, op0=ALU.add)
            yi = ysd.bitcast(I32)
            xi = rs.bitcast(I32)
            nc.vector.tensor_scalar(yi, xi, 1, None,
                                    op0=ALU.logical_shift_right)
            nc.vector.tensor_scalar(yi, yi, -1, 0x5F3759DF,
                                    op0=ALU.mult, op1=ALU.add)
            nc.vector.tensor_scalar(rs, rs, 0.5, None, op0=ALU.mult)
            nc.vector.tensor_tensor(cn, ysd, ysd, op=ALU.mult)
            nc.vector.tensor_tensor(cn, cn, rs, op=ALU.mult)
            nc.vector.tensor_scalar(cn, cn, -1.0, 1.5,
                                    op0=ALU.mult, op1=ALU.add)
            nc.vector.tensor_tensor(ysd, ysd, cn, op=ALU.mult)
            # g = nsq * rsqrt(nsq+eps) / (1+nsq)
            nc.vector.tensor_scalar(rs, nsq, 1.0, None, op0=ALU.add)
            nc.vector.reciprocal(rs, rs)
            nc.vector.tensor_tensor(ysd, ysd, nsq, op=ALU.mult)
            if fold_r0:
                nc.vector.tensor_tensor(ysd, ysd, rs, op=ALU.mult)
                r0_b = r0_t[:, h0:h1].unsqueeze(2).broadcast_to(
                    [128, h1 - h0, CO])
                nc.vector.tensor_tensor(out_g[:, h0:h1], ysd, r0_b,
                                        op=ALU.mult)
            else:
                nc.vector.tensor_tensor(out_g[:, h0:h1], ysd, rs,
                                        op=ALU.mult)

        # squash stat tree, split in two parts: sq on Act (emitted with its
        # chunk), the small DVE n-levels emitted later so they never stall
        # the big-product pipeline
        def nsq_sq(src_t, h0, h1):
            n = h1 - h0
            sq = scr.tile([128, n, DO, CO], BF16, name="sq", tag="SQ", bufs=3)
            nc.scalar.activation(sq[:], src_t[:, h0:h1], AF.Square)
            return sq

        def nsq_levels(sq, h0, h1):
            n = h1 - h0
            n1 = scr.tile([128, n, 8, CO], BF16, name="n1", tag="N1", bufs=1)
            nc.vector.tensor_tensor(n1[:], sq[:, :, 0:8], sq[:, :, 8:16],
                                    op=ALU.add)
            n2 = scr.tile([128, n, 4, CO], BF16, name="n2", tag="N2", bufs=1)
            nc.vector.tensor_tensor(n2[:], n1[:, :, 0:4], n1[:, :, 4:8],
                                    op=ALU.add)
            n3 = scr.tile([128, n, 2, CO], BF16, name="n3", tag="N3", bufs=1)
            nc.vector.tensor_tensor(n3[:], n2[:, :, 0:2], n2[:, :, 2:4],
                                    op=ALU.add)
            nc.vector.tensor_tensor(nsq_t[:, h0:h1], n3[:, :, 0], n3[:, :, 1],
                                    op=ALU.add)

        # big product + d-tree: db_t[:, h0:h1] = sum_d uhat * m_b  (m bf16
        # [128,HB,DO,CO] broadcast over ci)
        def d_contract(m_t, rcm, h0, h1):
            n = h1 - h0
            m_b = m_t[:].unsqueeze(2).broadcast_to([128, HB, CI, DO, CO])
            y = scr.tile([128, n, CI, DO, CO], BF16, name="y", tag="X")
            hb2(ALU.mult, h0, h1, 1,
                lambda a, b: y[:, a - h0 : b - h0],
                lambda a, b: uhat[:, a:b],
                lambda a, b: m_b[:, a:b])
            if h0 in PE_D:
                for g0 in range(0, n, 2):
                    g1 = min(g0 + 2, n)
                    dps = psum.tile([128, 2, CI, CO], F32, name="dps",
                                    tag="pr", bufs=2)
                    o = dps[:, 0 : g1 - g0]
                    pe_sum(o, [y[:, g0:g1, :, d] for d in range(DO)])
                    evac(db_t[:, h0 + g0 : h0 + g1], o,
                         sz=(g1 - g0) * 128.0)
                return
            e1 = scr.tile([128, n, CI, 8, CO], BF16, name="e1", tag="T1")
            hb2(ALU.add, h0, h1, 1,
                lambda a, b: e1[:, a - h0 : b - h0],
                lambda a, b: y[:, a - h0 : b - h0, :, 0:8],
                lambda a, b: y[:, a - h0 : b - h0, :, 8:16])
            e2 = scr.tile([128, n, CI, 4, CO], BF16, name="e2", tag="T2")
            hb2(ALU.add, h0, h1, 1,
                lambda a, b: e2[:, a - h0 : b - h0],
                lambda a, b: e1[:, a - h0 : b - h0, :, 0:4],
                lambda a, b: e1[:, a - h0 : b - h0, :, 4:8])
            e3 = scr.tile([128, n, CI, 2, CO], BF16, name="e3", tag="T3")
            hb2(ALU.add, h0, h1, 1,
                lambda a, b: e3[:, a - h0 : b - h0],
                lambda a, b: e2[:, a - h0 : b - h0, :, 0:2],
                lambda a, b: e2[:, a - h0 : b - h0, :, 2:4])
            hb2(ALU.add, h0, h1, 1,
                lambda a, b: db_t[:, a:b],
                lambda a, b: e3[:, a - h0 : b - h0, :, 0],
                lambda a, b: e3[:, a - h0 : b - h0, :, 1])

        # i-contract: p_t[:, h0:h1] = sum_i c_t * uhat  (c_t bf16 weights)
        def i_contract(h0, h1):
            n = h1 - h0
            r_b = c_t[:].unsqueeze(3).broadcast_to([128, HB, CI, DO, CO])
            x = scr.tile([128, n, CI, DO, CO], BF16, name="x", tag="X")
            hb2(ALU.mult, h0, h1, 1,
                lambda a, b: x[:, a - h0 : b - h0],
                lambda a, b: uhat[:, a:b],
                lambda a, b: r_b[:, a:b])
            if h0 in PE_I:
                for g0 in range(0, n, 2):
                    g1 = min(g0 + 2, n)
                    pps = psum.tile([128, 2, DO, CO], F32, name="pps",
                                    tag="pr", bufs=2)
                    o = pps[:, 0 : g1 - g0]
                    pe_sum(o, [x[:, g0:g1, i] for i in range(CI)])
                    evac(p_t[:, h0 + g0 : h0 + g1], o,
                         sz=(g1 - g0) * 256.0)
                return
            t1 = scr.tile([128, n, 4, DO, CO], BF16, name="t1", tag="T1")
            hb2(ALU.add, h0, h1, 1,
                lambda a, b: t1[:, a - h0 : b - h0],
                lambda a, b: x[:, a - h0 : b - h0, 0:4],
                lambda a, b: x[:, a - h0 : b - h0, 4:8])
            t2 = scr.tile([128, n, 2, DO, CO], BF16, name="t2", tag="T2")
            hb2(ALU.add, h0, h1, 1,
                lambda a, b: t2[:, a - h0 : b - h0],
                lambda a, b: t1[:, a - h0 : b - h0, 0:2],
                lambda a, b: t1[:, a - h0 : b - h0, 2:4])
            hb2(ALU.add, h0, h1, 1,
                lambda a, b: p_t[:, a:b],
                lambda a, b: t2[:, a - h0 : b - h0, 0],
                lambda a, b: t2[:, a - h0 : b - h0, 1])

        # b max-tree (over co) -> m0C chunk (+CC)
        def u_tree(h0, h1):
            n = h1 - h0
            u1 = scr.tile([128, n, CI, 8], F32, name="u1", tag="U1", bufs=1)
            nc.vector.tensor_tensor(
                u1[:], b_t[:, h0:h1, :, 0:8],
                b_t[:, h0:h1, :, 8:16], op=ALU.max)
            u2 = scr.tile([128, n, CI, 4], F32, name="u2", tag="U2", bufs=1)
            nc.vector.tensor_tensor(u2[:], u1[:, :, :, 0:4], u1[:, :, :, 4:8],
                                    op=ALU.max)
            u3 = scr.tile([128, n, CI, 2], F32, name="u3", tag="U3", bufs=1)
            nc.vector.tensor_tensor(u3[:], u2[:, :, :, 0:2], u2[:, :, :, 2:4],
                                    op=ALU.max)
            nc.vector.tensor_tensor(m0C_t[:, h0:h1], u3[:, :, :, 0],
                                    u3[:, :, :, 1], op=ALU.max)
            nc.vector.tensor_scalar(m0C_t[:, h0:h1], m0C_t[:, h0:h1], CC,
                                    None, op0=ALU.add)

        # =========== Stage 2 it0: b1 = (g0*r0) .* (sum_d uhat*S) ===========
        # big products first (only need S), squash stats run concurrently
        sqs = {}
        for (h0, h1) in CHUNKS:
            d_contract(S_t, None, h0, h1)   # y0 = uhat*S_b, tree -> db_t
            sqs[h0] = nsq_sq(S_t, h0, h1)   # Act squares, no DVE stall
        for (h0, h1) in CHUNKS:
            nsq_levels(sqs[h0], h0, h1)
        r0sq_b = r0sq_t[:].unsqueeze(2).broadcast_to([128, HB, CO])
        nc.vector.tensor_tensor(nsq_t[:], nsq_t[:], r0sq_b, op=ALU.mult)
        g_chain(g2r0_t, True, 0, HB)
        g_b0 = g2r0_t[:].unsqueeze(2).broadcast_to([128, HB, CI, CO])
        for (h0, h1) in CHUNKS:
            hb2(ALU.mult, h0, h1, 2,
                lambda a, b: b_t[:, a:b],
                lambda a, b: db_t[:, a:b],
                lambda a, b: g_b0[:, a:b])
            u_tree(h0, h1)

        # =========== Stage 2 it1/it2 ===========
        for it in (1, 2):
            last = it == ROUTING - 1
            pe_max_pools()
            # cs = (bmaxC - CC) - b = -(b - bmax); exp applies scale=-1
            nc.vector.tensor_scalar(
                tA[:, 0:18], m0C_t[:, 0:18], -CC, None, op0=ALU.add)
            cs = scr.tile([128, HB, CI, CO], F32, name="cs", tag="X")
            bm_b = tA[:, 0:18].unsqueeze(3).broadcast_to([128, HB, CI, CO])
            for (e0, e1_) in ((0, 6), (6, 12), (12, 18)):
                hb2(ALU.subtract, e0, e1_, 2,
                    lambda a, b: cs[:, a:b],
                    lambda a, b: bm_b[:, a:b],
                    lambda a, b: b_t[:, a:b])
                nc.scalar.activation(c_t[:, e0:e1_], cs[:, e0:e1_], AF.Exp,
                                     scale=-1.0)
            s1 = scr.tile([128, HB, CI, 8], BF16, name="s1", tag="S1")
            nc.vector.tensor_tensor(
                s1[:], c_t[:, :, :, 0:8], c_t[:, :, :, 8:16], op=ALU.add)
            s2 = scr.tile([128, HB, CI, 4], BF16, name="s2", tag="S2")
            nc.vector.tensor_tensor(
                s2[:], s1[:, :, :, 0:4], s1[:, :, :, 4:8], op=ALU.add)
            s3 = scr.tile([128, HB, CI, 2], BF16, name="s3", tag="S3")
            nc.vector.tensor_tensor(
                s3[:], s2[:, :, :, 0:2], s2[:, :, :, 2:4], op=ALU.add)
            nc.vector.tensor_tensor(
                s_tb[:, 0:18], s3[:, :, :, 0], s3[:, :, :, 1], op=ALU.add)
            pe_sum_pools()
            nc.vector.reciprocal(rcp_t[:], sumc_t[:])
            nc.vector.tensor_copy(rcpb_t[:], rcp_t[:])
            rb_ = rcpb_t[:].unsqueeze(3).broadcast_to([128, HB, CI, CO])
            hb2(ALU.mult, 0, HB, 4,
                lambda a, b: c_t[:, a:b],
                lambda a, b: c_t[:, a:b],
                lambda a, b: rb_[:, a:b])
            # p = sum_i r*uhat ; squash stats per chunk on Act
            sqs = {}
            for (h0, h1) in CHUNKS:
                i_contract(h0, h1)
                sqs[h0] = nsq_sq(p_t, h0, h1)
            if not last:
                for (h0, h1) in CHUNKS:
                    d_contract(p_t, None, h0, h1)
                for (h0, h1) in CHUNKS:
                    nsq_levels(sqs[h0], h0, h1)
                g_chain(g2b_t, False, 0, HB)
                g_b1 = g2b_t[:].unsqueeze(2).broadcast_to(
                    [128, HB, CI, CO])
                for (h0, h1) in CHUNKS:
                    hb2(ALU.mult, h0, h1, 1,
                        lambda a, b: db_t[:, a:b],
                        lambda a, b: db_t[:, a:b],
                        lambda a, b: g_b1[:, a:b])
                    hb2(ALU.add, h0, h1, 2,
                        lambda a, b: b_t[:, a:b],
                        lambda a, b: b_t[:, a:b],
                        lambda a, b: db_t[:, a:b])
                    u_tree(h0, h1)
            else:
                for (hh0, hh1) in ((0, 10), (10, 18)):
                    for (h0, h1) in CHUNKS:
                        if h0 < hh0 or h1 > hh1:
                            continue
                        nsq_levels(sqs[h0], h0, h1)
                    g_chain(g2b_t, False, hh0, hh1)
                    for (h0, h1) in CHUNKS:
                        if h0 < hh0 or h1 > hh1:
                            continue
                        g_b2 = g2b_t[:].unsqueeze(2).broadcast_to(
                            [128, HB, DO, CO])
                        hb2(ALU.mult, h0, h1, 1,
                            lambda a, b: v_bf[:, a:b],
                            lambda a, b: p_t[:, a:b],
                            lambda a, b: g_b2[:, a:b])
                        nc.sync.dma_start(v_d[:, h0:h1], v_bf[:, h0:h1])
    return nc


# ============================ host side ============================

_CACHE = {}


def _host_consts(w):
    # w: [Ci, Co*Do, Di, 5, 5] f32, channel index = co*16+do.
    # Conv lhsT rows: pat1 row = di*16 + kh*4 + kw (kh,kw in 0..4);
    # pat2 row = di*5 + kw for (kh=4, kw 0..5), then 40 + di*4 + kh for
    # (kh 0..4, kw=4).  Columns m = do*16 + co.
    w4 = w.reshape(CI, CO, DO, DI, 5, 5).transpose(3, 4, 5, 0, 2, 1)
    # w4: [di, kh, kw, ci, do, co]
    w4 = np.ascontiguousarray(w4).reshape(DI, 5, 5, CI, 256)
    w1 = np.ascontiguousarray(
        w4[:, 0:4, 0:4].reshape(128, CI, 256)
    ).astype(ml_dtypes.bfloat16)
    w2a = w4[:, 4, 0:5].reshape(40, CI, 256)
    w2b = w4[:, 0:4, 4].reshape(32, CI, 256)
    w2 = np.ascontiguousarray(np.concatenate([w2a, w2b], 0)).astype(
        ml_dtypes.bfloat16
    )

    hw_cnt = np.zeros((H, W), np.float32)
    for h in range(H):
        for wv in range(W):
            ch = min(h + 2, H - 1) - max(h - 2, 0) + 1
            cw = min(wv + 2, W - 1) - max(wv - 2, 0) + 1
            hw_cnt[h, wv] = ch * cw
    r0 = 1.0 / (CO * hw_cnt)
    r0c = np.ascontiguousarray(r0.reshape(HB, 128).T)
    return w1, w2, r0c


def _shift_mats():
    """Constant PE shift matrices for the 5x5 window pools (see _emit)."""
    def wof(cls, p):
        return (32 * cls + p) % 48

    smf = np.zeros((128, 8, 128), np.float32)
    for cls in range(3):
        for m in range(128):
            w = wof(cls, m)
            if m + 1 < 128 and w < 47:
                smf[m + 1, cls, m] = 1.0
            if m - 1 >= 0 and w >= 1:
                smf[m - 1, 3 + cls, m] = 1.0
    for m in range(80):
        smf[m + 48, 6, m] = 1.0
    for m in range(48, 128):
        smf[m - 48, 7, m] = 1.0

    carw = np.zeros((2, 2, 128), np.float32)
    carw[0, 0, 127] = 1.0  # c1p cls0 (w(127)=31 valid)
    carw[0, 1, 127] = 1.0  # c1p cls1 (w(127)=15 valid)

    carf = np.zeros((128, 3, 128), np.float32)
    carf[127, 0, 0] = 1.0  # c1m cls1 (w(0)=32 valid)
    carf[127, 1, 0] = 1.0  # c1m cls2 (w(0)=16 valid)
    for m in range(48):    # chm: out[m] = in_prev[m+80]
        carf[m + 80, 2, m] = 1.0

    chp = np.zeros((48, 128), np.float32)
    for m in range(80, 128):
        chp[m - 80, m] = 1.0

    smb = np.zeros((128, 4, 128), np.float32)
    for cls in range(3):
        for m in range(128):
            for dw in range(-2, 3):
                k = m + dw
                if 0 <= k < 128 and 0 <= wof(cls, m) + dw < 48:
                    smb[k, cls, m] = 1.0
    for m in range(128):
        for dh in range(-2, 3):
            k = m + 48 * dh
            if 0 <= k < 128:
                smb[k, 3, m] = 1.0

    cwp = np.zeros((2, 3, 128), np.float32)
    carb = np.zeros((128, 2, 128), np.float32)
    for cls in range(3):
        for m in range(126, 128):
            for dw in (1, 2):
                k = m + dw - 128
                if 0 <= k < 2 and wof(cls, m) + dw < 48:
                    cwp[k, cls, m] = 1.0
        if cls > 0:
            for m in range(0, 2):
                for dw in (-2, -1):
                    k = m + dw + 128
                    if 126 <= k < 128 and wof(cls, m) + dw >= 0:
                        carb[k, cls - 1, m] = 1.0

    chs = np.zeros((128, 2, 128), np.float32)
    for m in range(80, 128):
        chs[m - 80, 0, m] += 1.0
    for m in range(32, 128):
        chs[m - 32, 0, m] += 1.0
    for m in range(48):
        chs[m + 80, 1, m] += 1.0
    for m in range(96):
        chs[m + 32, 1, m] += 1.0

    bf = ml_dtypes.bfloat16
    return (smf, carw, carf, chp, smb.astype(bf), cwp.astype(bf),
            carb.astype(bf), chs.astype(bf))


def _im2col(un):
    """un: [Ci, Di, H, W] bf16 -> pat1 [Ci, 128, HW], pat2 [Ci, 72, HW] bf16.
    Row layouts match _host_consts."""
    up = np.zeros((CI, DI, H + 4, W + 4), ml_dtypes.bfloat16)
    up[:, :, 2 : 2 + H, 2 : 2 + W] = un
    sw = np.lib.stride_tricks.sliding_window_view(up, (H, W), axis=(2, 3))
    # sw: [Ci, Di, 5, 5, H, W]
    p1 = sw[:, :, 0:4, 0:4].reshape(CI, DI * 16, HW)
    p2a = sw[:, :, 4, 0:5].reshape(CI, DI * 5, HW)
    p2b = sw[:, :, 0:4, 4].reshape(CI, DI * 4, HW)
    p2 = np.concatenate([p2a, p2b], 1)
    return np.ascontiguousarray(p1), np.ascontiguousarray(p2)


def _get_nc():
    if "nc" not in _CACHE:
        nc = bacc.Bacc("TRN2", target_bir_lowering=False, debug=False, num_devices=8)
        _emit(nc)
        nc.compile()
        _CACHE["nc"] = nc
    return _CACHE["nc"]


def kernel(u, w):
    u = np.asarray(u, np.float32)
    N = u.shape[0]
    assert N == 8
    nc = _get_nc()
    w1, w2, r0c = _host_consts(np.asarray(w, np.float32))
    smf, carw, carf, chp, smb, cwp, carb, chs = _shift_mats()
    idn = np.eye(128, dtype=ml_dtypes.bfloat16)
    ub = u.astype(ml_dtypes.bfloat16)
    in_maps = []
    for n in range(N):
        p1, p2 = _im2col(ub[n])
        in_maps.append({"p1": p1, "p2": p2, "w1": w1, "w2": w2, "r0c": r0c,
                        "smf": smf, "carw": carw, "carf": carf, "chp": chp,
                        "smb": smb, "cwp": cwp, "carb": carb, "chs": chs,
                        "idn": idn})
    res = run_bass_kernel_spmd(nc, in_maps, core_ids=list(range(N)))
    out = np.stack(
        [res.results[n]["v"].astype(np.float32) for n in range(N)]
    )  # [8, 128, HB, DO, CO]
    # hw = hb*128 + p ; out[n, co, do, h, w]
    out = out.transpose(0, 2, 1, 3, 4).reshape(N, HW, DO, CO)
    out = out.reshape(N, H, W, DO, CO).transpose(0, 4, 3, 1, 2)
    return np.ascontiguousarray(out, dtype=np.float32)
